# revision 2
# baseline (speedup 1.0000x reference)
"""Distributed causal attention block (QKV + RoPE + SDPA + Wo) on 8 TRN2 cores.

Sharding: tensor-parallel over heads (2 heads/core). Each core:
  phase 1: weight-stationary transposed QKV: q^T/k^T/v^T = Wqkv_c @ x^T
           streamed from host-pretransposed x^T (bf16); RoPE applied in the
           [e, t] layout with per-head even/odd partition split; v^T is
           PE-transposed back to [t, e] for the PV matmuls
  phase 2: causal attention per (batch, head) with TRANSPOSED scores
           s^T[k, q], q-chunk-major: each 512-token q-chunk computes its
           k-blocks in order, exp's them in bank-pairs (wide ACT ops), and
           immediately feeds PV PSUM-accumulation (pair-lagged) plus softmax
           sum accumulation (h0 on Pool, h1 on DVE); partition-sum via a
           single all-ones matmul per chunk-head; 1/sum via DVE reciprocal
  phase 3: AllGather attention outputs (bounced per-chunk across DMA queues)
           -> Wo e-slice, interleaved with phase 2(b=1) to avoid a tail
Host concatenates the 8 e-slices.

The q/k rows of Wqkv (and cos/sin tables) are permuted head-major
even/odd on the host; attention scores are invariant to a shared
permutation of the head dim of Q and K.
"""
import numpy as np
import ml_dtypes
import bass_rust
import concourse.bass as bass
import concourse.mybir as mybir
from concourse.tile import TileContext
from concourse.masks import make_identity

B, L, D, H = 2, 2048, 2048, 16
HD = 128
N_CORES = 8
HPC = H // N_CORES          # heads per core = 2
ES = HPC * HD               # 256 = e-slice width per core
T = B * L                   # 4096 tokens total
P = 128
SCALE = 1.0 / float(np.sqrt(HD))
FP = mybir.dt.float32
BF = mybir.dt.bfloat16

N_TT = T // P               # 32 global t-tiles
N_LT = L // P               # 16 t-tiles per batch
N_DT = D // P               # 16 d-tiles

# attention-out AllGather pieces per batch, in units of 512-t q-chunks
AG_PIECES = {0: [(0, 4)], 1: [(0, 2), (2, 4)]}


def piece_of(b, qc):
    for (c0, c1) in AG_PIECES[b]:
        if c0 <= qc < c1:
            return (c0, c1)
    raise AssertionError


def chunk_blocks(qc):
    """k-blocks of q-chunk qc: (kt, off, w, scol) in kt order.
    off = column offset within the chunk (fully-masked prefix), w = width,
    scol = column offset of the block in the chunk's S tile."""
    blks = []
    scol = 0
    for kt in range(4 * qc + 4):
        off = max(0, kt * 128 - qc * 512)
        w = 512 - off
        blks.append((kt, off, w, scol))
        scol += w
    return blks


def split_multi_waits(nc):
    """This walrus build allows 1 sync wait per instruction (2 for
    EventSemaphore). Tile attaches more on some instructions (tail drain,
    collective-adjacent DMAs); hoist the extras onto same-engine NoOps."""
    for f in nc.m.functions:
        for bb in f.blocks:
            new_insts = []
            changed = False
            for ins in bb.instructions:
                si = ins.sync_info
                cap = 2 if type(ins).__name__ == "InstEventSemaphore" else 1
                if si is not None and len(si.on_wait) > cap:
                    waits = list(si.on_wait)
                    for k, w in enumerate(waits[cap:]):
                        new_insts.append(mybir.InstNoOp(
                            name=f"{ins.name}-wsplit{k}", ins=[], outs=[],
                            engine=ins.engine,
                            sync_info=bass_rust.SyncInfo(on_wait=[w], on_update=[]),
                        ))
                    ins.sync_info = bass_rust.SyncInfo(
                        on_wait=waits[:cap], on_update=list(si.on_update))
                    changed = True
                new_insts.append(ins)
            if changed:
                bb.instructions.clear()
                for i2 in new_insts:
                    bb.add_instruction(i2)


def build(fix_waits=True, dummy_cc=True):
    nc = bass.Bass()
    xT = nc.declare_dram_parameter("xT", [D, T], BF, isOutput=False)
    wqkvT = nc.declare_dram_parameter("wqkvT", [D, 3 * ES], BF, isOutput=False)
    # per-head stacked trig tables: rows 0:64 = even-col table, 64:128 = odd
    cc_p = [nc.declare_dram_parameter(f"cc{h}", [P, L], BF, isOutput=False)
            for h in range(HPC)]
    ss_p = [nc.declare_dram_parameter(f"ss{h}", [P, L], BF, isOutput=False)
            for h in range(HPC)]
    woT = nc.declare_dram_parameter("woT", [D, ES], BF, isOutput=False)
    out = nc.declare_dram_parameter("out", [ES, T], BF, isOutput=True)

    o_bounce, ag_o = {}, {}
    for b, pieces in AG_PIECES.items():
        for (c0, c1) in pieces:
            w = (c1 - c0) * 512
            o_bounce[(b, c0)] = nc.dram_tensor(f"o_bounce{b}_{c0}", [ES, w], BF)
            ag_o[(b, c0)] = nc.dram_tensor(f"ag_o{b}_{c0}", [N_CORES * ES, w], BF,
                                           addr_space="Shared")
    rg = [list(range(N_CORES))]
    if dummy_cc:
        dummy_in = nc.dram_tensor("dummy_in", [1, 256], BF)
        dummy_out = nc.dram_tensor("dummy_ag", [N_CORES, 256], BF,
                                   addr_space="Shared")

    with TileContext(nc, pool_alloc_mode="queue") as tc:
        with (
            tc.tile_pool(name="const", bufs=1) as const_pool,
            tc.tile_pool(name="resident", bufs=1) as res_pool,
            tc.tile_pool(name="wo", bufs=1) as wo_pool,
            tc.tile_pool(name="vt", bufs=1) as vt_pool,
        ):
            if dummy_cc:
                # skew-absorbing tiny collective: aligns the 8 cores while
                # phase 1 computes, so the real AllGathers don't eat the skew
                zt = const_pool.tile([1, 256], BF, name="zt")
                nc.gpsimd.memset(zt[:, :], 0.0)
                nc.gpsimd.dma_start(out=dummy_in[:, :], in_=zt[:, :])
                nc.gpsimd.collective_compute(
                    "AllGather", mybir.AluOpType.bypass,
                    ins=[dummy_in[:]], outs=[dummy_out[:]],
                    replica_groups=rg)

            ident = const_pool.tile([P, P], BF, name="ident")
            make_identity(nc, ident[:, :])
            ones = const_pool.tile([P, P], BF, name="ones")
            nc.gpsimd.memset(ones[:, :], 1.0)
            tri = const_pool.tile([P, P], BF, name="tri")
            nc.gpsimd.memset(tri[:, :], 1.0)
            nc.gpsimd.affine_select(
                out=tri[:, :], in_=tri[:, :],
                compare_op=mybir.AluOpType.is_ge, fill=0.0, base=0,
                pattern=[[1, P]], channel_multiplier=-1)

            # resident through phases 1-2
            qt_sb = res_pool.tile([P, HPC * T], BF, name="qt_sb")   # [hd', h*T+t]
            kt_sb = res_pool.tile([P, HPC * T], BF, name="kt_sb")
            v_sb = res_pool.tile([P, N_TT * ES], BF, name="v_sb")   # [t%128, tt*ES+e]
            woT_sb = wo_pool.tile([P, N_DT * ES], BF, name="woT_sb")

            # ---------------- phase 1: transposed QKV + RoPE ----------------
            # eb order: q-h0, q-h1, k-h0, k-h1, v-0, v-1
            with (
                tc.tile_pool(name="wq", bufs=1) as wq_pool,
                tc.tile_pool(name="xt", bufs=1) as xt_pool,
                tc.tile_pool(name="rsc", bufs=1) as rsc_pool,
                tc.tile_pool(name="psG", bufs=2, space="PSUM") as psG,
            ):
                wt_sb = wq_pool.tile([P, N_DT * 3 * ES], BF, name="wt_sb")
                cc_sb = [wq_pool.tile([P, L], BF, name=f"cc{h}_sb")
                         for h in range(HPC)]
                ss_sb = [wq_pool.tile([P, L], BF, name=f"ss{h}_sb")
                         for h in range(HPC)]
                xt_sb = xt_pool.tile([P, N_DT * 2048], BF, name="xt_sb")
                vt_sb = vt_pool.tile([P, HPC * T], BF,
                                     name="vt_sb")  # [e, eb*T + t]

                # DMA priority: first tiles split small so the first matmul
                # can start early; then x^T/weights interleaved, then trig
                for dt in range(N_DT):
                    if dt < 2:
                        for c in range(4):
                            nc.sync.dma_start(
                                out=xt_sb[:, dt * 2048 + c * 512:
                                          dt * 2048 + (c + 1) * 512],
                                in_=xT[dt * P:(dt + 1) * P,
                                       c * 512:(c + 1) * 512])
                            if c == 0:
                                nc.sync.dma_start(
                                    out=wt_sb[:, dt * 3 * ES:(dt + 1) * 3 * ES],
                                    in_=wqkvT[dt * P:(dt + 1) * P, :])
                    else:
                        nc.sync.dma_start(
                            out=xt_sb[:, dt * 2048:(dt + 1) * 2048],
                            in_=xT[dt * P:(dt + 1) * P, 0:2048])
                        nc.sync.dma_start(
                            out=wt_sb[:, dt * 3 * ES:(dt + 1) * 3 * ES],
                            in_=wqkvT[dt * P:(dt + 1) * P, :])
                for h in range(HPC):
                    nc.sync.dma_start(out=cc_sb[h][:, :], in_=cc_p[h][:, :])
                    nc.sync.dma_start(out=ss_sb[h][:, :], in_=ss_p[h][:, :])

                def rope_drain(gp, dst, h, th):
                    cc, ss = cc_sb[h], ss_sb[h]
                    dcol = slice(h * T + th * 2048, h * T + (th + 1) * 2048)
                    e_ps, o_ps = gp[0:64, :], gp[64:128, :]
                    t1 = rsc_pool.tile([64, 2048], FP, name="t1", tag="t1")
                    t2 = rsc_pool.tile([64, 2048], FP, name="t2", tag="t2")
                    nc.vector.tensor_tensor(t1[:, :], e_ps, cc[0:64, :],
                                            op=mybir.AluOpType.mult)
                    nc.vector.tensor_tensor(t2[:, :], o_ps, ss[0:64, :],
                                            op=mybir.AluOpType.mult)
                    nc.vector.tensor_tensor(dst[0:64, dcol], t1[:, :], t2[:, :],
                                            op=mybir.AluOpType.subtract)
                    t3 = rsc_pool.tile([64, 2048], FP, name="t3", tag="t1")
                    t4 = rsc_pool.tile([64, 2048], FP, name="t4", tag="t2")
                    nc.vector.tensor_tensor(t3[:, :], o_ps, cc[64:128, :],
                                            op=mybir.AluOpType.mult)
                    nc.vector.tensor_tensor(t4[:, :], e_ps, ss[64:128, :],
                                            op=mybir.AluOpType.mult)
                    nc.vector.tensor_tensor(dst[64:128, dcol], t3[:, :], t4[:, :],
                                            op=mybir.AluOpType.add)

                # th0: v first (tables can lag); th1: end with v so the
                # transpose stage isn't gated on a RoPE drain
                EB_ORDERS = [[4, 5, 0, 1, 2, 3], [4, 0, 1, 2, 3, 5]]
                for th in range(2):
                    for i, ebi in enumerate(EB_ORDERS[th]):
                        gp = psG.tile([P, 2048], FP, name="gp", tag="gp")
                        for dt in range(N_DT):
                            lhsT = wt_sb[:, dt * 3 * ES + ebi * P:
                                         dt * 3 * ES + (ebi + 1) * P]
                            for c in range(4):
                                nc.tensor.matmul(
                                    gp[:, c * 512:(c + 1) * 512], lhsT,
                                    xt_sb[:, dt * 2048 + c * 512:
                                          dt * 2048 + (c + 1) * 512],
                                    start=(dt == 0), stop=(dt == N_DT - 1))
                            if th == 0 and i == 5:
                                nc.sync.dma_start(
                                    out=xt_sb[:, dt * 2048:(dt + 1) * 2048],
                                    in_=xT[dt * P:(dt + 1) * P, 2048:4096])
                        if ebi < 2:
                            rope_drain(gp, qt_sb, ebi, th)
                        elif ebi < 4:
                            rope_drain(gp, kt_sb, ebi - 2, th)
                        else:
                            eb2 = ebi - 4
                            nc.scalar.copy(
                                vt_sb[:, eb2 * T + th * 2048:
                                      eb2 * T + (th + 1) * 2048],
                                gp[:, :])

            # ---------------- phases 2+3 (interleaved) ----------------
            for dt in range(N_DT):
                nc.sync.dma_start(out=woT_sb[:, dt * ES:(dt + 1) * ES],
                                  in_=woT[dt * P:(dt + 1) * P, :])

            with (
                tc.tile_pool(name="pS", bufs=2) as pS,
                tc.tile_pool(name="pAcc", bufs=2) as pAcc,
                tc.tile_pool(name="p2ob", bufs=2) as p2ob,
                tc.tile_pool(name="p3x", bufs=2) as p3x,
                tc.tile_pool(name="p3o", bufs=2) as p3o,
                tc.tile_pool(name="psW", bufs=3, space="PSUM") as psW,
                tc.tile_pool(name="psO", bufs=1, space="PSUM") as psO,
            ):
                sum_eng = {0: nc.gpsimd, 1: nc.vector}

                def phase2_chunk(b, qc, ob_sb):
                    blks = chunk_blocks(qc)
                    pairs = [tuple(blks[i:i + 2])
                             for i in range(0, len(blks), 2)]
                    CW = blks[-1][3] + blks[-1][2]
                    nkt = 4 * qc + 4
                    S = {h: pS.tile([P, CW], BF, name=f"S{h}", tag=f"S{h}")
                         for h in range(HPC)}
                    acc = {h: pAcc.tile([P, 512], FP, name=f"acc{h}",
                                        tag=f"acc{h}")
                           for h in range(HPC)}
                    o_ps = {h: psO.tile([P, 512], FP, name=f"o{h}",
                                        tag=f"o{h}")
                            for h in range(HPC)}

                    def pv_pair(pr, h):
                        for (kt, off, w, scol) in pr:
                            nc.tensor.matmul(
                                o_ps[h][:, off:],
                                v_sb[:, (b * N_LT + kt) * ES + h * HD:
                                     (b * N_LT + kt) * ES + (h + 1) * HD],
                                S[h][:, scol:scol + w],
                                start=(kt == 0), stop=(kt == nkt - 1))

                    for i, pr in enumerate(pairs):
                        for h in range(HPC):
                            qoff = h * T + b * L
                            sp = psW.tile([P, 1024], FP, name="sp", tag="w")
                            spo = 0
                            for (kt, off, w, scol) in pr:
                                nc.tensor.matmul(
                                    sp[:, spo:spo + w],
                                    kt_sb[:, qoff + kt * P:
                                          qoff + (kt + 1) * P],
                                    qt_sb[:, qoff + qc * 512 + off:
                                          qoff + (qc + 1) * 512],
                                    start=True, stop=True)
                                spo += w
                            wtot = sum(pr_b[2] for pr_b in pr)
                            nc.scalar.activation(
                                S[h][:, pr[0][3]:pr[0][3] + wtot],
                                sp[:, 0:wtot],
                                mybir.ActivationFunctionType.Exp, scale=SCALE)
                            # diagonal blocks: zero the masked (k>q) triangle
                            for (kt, off, w, scol) in pr:
                                if kt >= 4 * qc:
                                    sum_eng[h].tensor_tensor(
                                        S[h][:, scol:scol + P],
                                        S[h][:, scol:scol + P],
                                        tri[:, :], op=mybir.AluOpType.mult)
                            # softmax sum accumulation (h0: Pool, h1: DVE)
                            for (kt, off, w, scol) in pr:
                                if kt == 0:
                                    sum_eng[h].tensor_copy(
                                        acc[h][:, :], S[h][:, 0:512])
                                else:
                                    sum_eng[h].tensor_tensor(
                                        acc[h][:, off:], acc[h][:, off:],
                                        S[h][:, scol:scol + w],
                                        op=mybir.AluOpType.add)
                        if i > 0:
                            for h in range(HPC):
                                pv_pair(pairs[i - 1], h)
                    for h in range(HPC):
                        pv_pair(pairs[-1], h)

                    # finalize: partition-sum via all-ones matmul, reciprocal,
                    # rescale drain, per-chunk bounce DMA
                    accb, sm, rec = {}, {}, {}
                    for h in range(HPC):
                        accb[h] = pAcc.tile([P, 512], BF, name=f"accb{h}",
                                            tag=f"accb{h}")
                        sum_eng[h].tensor_copy(accb[h][:, :], acc[h][:, :])
                    for h in range(HPC):
                        sm[h] = psW.tile([P, 1024], FP, name=f"sm{h}", tag="w")
                        nc.tensor.matmul(sm[h][:, 0:512], ones[:, :],
                                         accb[h][:, :], start=True, stop=True)
                    for h in range(HPC):
                        rec[h] = pAcc.tile([P, 512], FP, name=f"rec{h}",
                                           tag=f"rec{h}")
                        nc.vector.reciprocal(rec[h][:, :], sm[h][:, 0:512])
                    (c0, c1) = piece_of(b, qc)
                    for h in range(HPC):
                        nc.vector.tensor_tensor(
                            ob_sb[:, h * L + qc * 512:h * L + (qc + 1) * 512],
                            o_ps[h][:, :], rec[h][:, :],
                            op=mybir.AluOpType.mult)
                        nc.sync.dma_start(
                            out=o_bounce[(b, c0)][h * HD:(h + 1) * HD,
                                                  (qc - c0) * 512:
                                                  (qc - c0 + 1) * 512],
                            in_=ob_sb[:, h * L + qc * 512:
                                      h * L + (qc + 1) * 512])

                def ag_fire(b, c0):
                    nc.gpsimd.collective_compute(
                        "AllGather", mybir.AluOpType.bypass,
                        ins=[o_bounce[(b, c0)][:]],
                        outs=[ag_o[(b, c0)][:]],
                        replica_groups=rg)

                def tr_group(th, eb2, tg):
                    tr = psW.tile([P, 512], BF, name="tr", tag="w")
                    for j in range(4):
                        tt_g = th * N_LT + tg * 4 + j
                        nc.tensor.transpose(
                            tr[:, j * P:(j + 1) * P],
                            vt_sb[:, eb2 * T + tt_g * P:
                                  eb2 * T + (tt_g + 1) * P],
                            ident[:, :])
                    for j in range(4):
                        tt_g = th * N_LT + tg * 4 + j
                        nc.vector.tensor_copy(
                            v_sb[:, tt_g * ES + eb2 * P:
                                 tt_g * ES + (eb2 + 1) * P],
                            tr[:, j * P:(j + 1) * P])

                def p3_load(b, c0, tch):
                    ot = p3x.tile([P, N_DT * 512], BF, name="ot", tag="ot")
                    for dt in range(N_DT):
                        nc.sync.dma_start(
                            out=ot[:, dt * 512:(dt + 1) * 512],
                            in_=ag_o[(b, c0)][dt * P:(dt + 1) * P,
                                              (tch - c0) * 512:
                                              (tch - c0 + 1) * 512])
                    return ot

                def p3_mm(b, tch, ot):
                    t0 = b * L + tch * 512
                    for et in range(HPC):
                        f_ps = psW.tile([P, 1024], FP, name="f_ps", tag="w")
                        for dt in range(N_DT):
                            nc.tensor.matmul(
                                f_ps[:, 0:512],
                                woT_sb[:, dt * ES + et * P:
                                       dt * ES + (et + 1) * P],
                                ot[:, dt * 512:(dt + 1) * 512],
                                start=(dt == 0), stop=(dt == N_DT - 1))
                        f_sb = p3o.tile([P, 512], BF, name="f_sb", tag="f")
                        nc.vector.tensor_copy(f_sb[:, :], f_ps[:, 0:512])
                        nc.sync.dma_start(
                            out=out[et * P:(et + 1) * P, t0:t0 + 512],
                            in_=f_sb[:, :])

                # ---- block 1: phase2(b=0), v transposes interleaved ----
                ob_tiles = {}
                for tg in range(4):           # batch-0 v tiles first
                    tr_group(0, 0, tg)
                    tr_group(0, 1, tg)
                ob_tiles[0] = p2ob.tile([P, HPC * L], BF,
                                        name="ob_sb", tag="ob")
                for qc in range(4):
                    phase2_chunk(0, qc, ob_tiles[0])
                    tr_group(1, 0, qc)        # batch-1 v tiles, spread out
                    tr_group(1, 1, qc)
                ag_fire(0, 0)

                # ---- block 2: phase2(b=1) with Wo pieces interleaved ----
                ob_tiles[1] = p2ob.tile([P, HPC * L], BF,
                                        name="ob_sb", tag="ob")
                phase2_chunk(1, 0, ob_tiles[1])
                phase2_chunk(1, 1, ob_tiles[1])
                ag_fire(1, 0)
                ot00 = p3_load(0, 0, 0)
                ot01 = p3_load(0, 0, 1)
                phase2_chunk(1, 2, ob_tiles[1])
                p3_mm(0, 0, ot00)
                p3_mm(0, 1, ot01)
                ot02 = p3_load(0, 0, 2)
                ot03 = p3_load(0, 0, 3)
                phase2_chunk(1, 3, ob_tiles[1])
                ag_fire(1, 2)
                p3_mm(0, 2, ot02)
                p3_mm(0, 3, ot03)
                for tch in (0, 1):
                    ot = p3_load(1, 0, tch)
                    p3_mm(1, tch, ot)
                for tch in (2, 3):
                    ot = p3_load(1, 2, tch)
                    p3_mm(1, tch, ot)

    if fix_waits:
        split_multi_waits(nc)
    return nc


def make_in_maps(x, cos, sin, Wqkv, Wo):
    bf = ml_dtypes.bfloat16
    xT_full = np.ascontiguousarray(
        np.asarray(x).reshape(T, D).T).astype(bf)
    # q/k row permutation: head-major, evens then odds
    perm = []
    for h in range(HPC):
        perm.extend(h * HD + 2 * np.arange(64))
        perm.extend(h * HD + 2 * np.arange(64) + 1)
    perm = np.asarray(perm)
    in_maps = []
    cosA, sinA = np.asarray(cos), np.asarray(sin)
    for c in range(N_CORES):
        cols = slice(c * ES, (c + 1) * ES)
        wq = Wqkv[c * ES:(c + 1) * ES, :][perm]
        wk = Wqkv[D + c * ES: D + (c + 1) * ES, :][perm]
        wv = Wqkv[2 * D + c * ES: 2 * D + (c + 1) * ES, :]
        w_c = np.concatenate([wq, wk, wv], axis=0)
        m = {
            "xT": xT_full,
            "wqkvT": np.ascontiguousarray(w_c.T.astype(bf)),
            "woT": np.ascontiguousarray(Wo[cols, :].T.astype(bf)),
        }
        for h in range(HPC):
            base = c * ES + h * HD
            ce = cosA[:, base + 2 * np.arange(64)].T      # [64, L]
            co = cosA[:, base + 2 * np.arange(64) + 1].T
            se = sinA[:, base + 2 * np.arange(64)].T
            so = sinA[:, base + 2 * np.arange(64) + 1].T
            m[f"cc{h}"] = np.ascontiguousarray(
                np.concatenate([ce, co], axis=0)).astype(bf)
            m[f"ss{h}"] = np.ascontiguousarray(
                np.concatenate([se, so], axis=0)).astype(bf)
        in_maps.append(m)
    return in_maps


def gather_out(res):
    pieces = [np.asarray(res.results[c]["out"]).astype(np.float32).T
              for c in range(N_CORES)]
    return np.concatenate(pieces, axis=1).reshape(B, L, D)


_cache = {}


def kernel(x, cos, sin, Wqkv, Wo):
    from concourse.bass_utils import run_bass_kernel_spmd
    x = np.asarray(x, dtype=np.float32)
    cos = np.asarray(cos, dtype=np.float32)
    sin = np.asarray(sin, dtype=np.float32)
    Wqkv = np.asarray(Wqkv, dtype=np.float32)
    Wo = np.asarray(Wo, dtype=np.float32)
    if "nc" not in _cache:
        _cache["nc"] = build()
    nc = _cache["nc"]
    in_maps = make_in_maps(x, cos, sin, Wqkv, Wo)
    res = run_bass_kernel_spmd(nc, in_maps, core_ids=list(range(N_CORES)))
    return gather_out(res)


# revision 4
# speedup vs baseline: 1.1542x; 1.1542x over previous
"""Distributed causal attention block (QKV + RoPE + SDPA + Wo) on 8 TRN2 cores.

Sharding: tensor-parallel over heads (2 heads/core). Each core:
  phase 1: weight-stationary transposed QKV: q^T/k^T/v^T = Wqkv_c @ x^T
           streamed from host-pretransposed x^T (bf16); RoPE applied in the
           [e, t] layout with per-head even/odd partition split; v^T is
           PE-transposed back to [t, e] for the PV matmuls
  phase 2: causal attention per (batch, head) with TRANSPOSED scores
           s^T[k, q], q-chunk-major: each 512-token q-chunk computes its
           k-blocks in order, exp's them in bank-pairs (wide ACT ops), and
           immediately feeds PV PSUM-accumulation (pair-lagged) plus softmax
           sum accumulation (h0 on Pool, h1 on DVE); partition-sum via a
           single all-ones matmul per chunk-head; 1/sum via DVE reciprocal
  phase 3: AllGather attention outputs (bounced per-chunk across DMA queues)
           -> Wo e-slice, interleaved with phase 2(b=1) to avoid a tail
Host concatenates the 8 e-slices.

The q/k rows of Wqkv (and cos/sin tables) are permuted head-major
even/odd on the host; attention scores are invariant to a shared
permutation of the head dim of Q and K.
"""
import numpy as np
import ml_dtypes
import bass_rust
import concourse.bass as bass
import concourse.mybir as mybir
from concourse.tile import TileContext
from concourse.masks import make_identity

B, L, D, H = 2, 2048, 2048, 16
HD = 128
N_CORES = 8
HPC = H // N_CORES          # heads per core = 2
ES = HPC * HD               # 256 = e-slice width per core
T = B * L                   # 4096 tokens total
P = 128
SCALE = 1.0 / float(np.sqrt(HD))
FP = mybir.dt.float32
BF = mybir.dt.bfloat16

N_TT = T // P               # 32 global t-tiles
N_LT = L // P               # 16 t-tiles per batch
N_DT = D // P               # 16 d-tiles

# attention-out AllGather pieces per batch, in units of 512-t q-chunks
AG_PIECES = {0: [(0, 4)], 1: [(0, 2), (2, 4)]}


def piece_of(b, qc):
    for (c0, c1) in AG_PIECES[b]:
        if c0 <= qc < c1:
            return (c0, c1)
    raise AssertionError


def chunk_blocks(qc):
    """k-blocks of q-chunk qc: (kt, off, w, scol) in kt order.
    off = column offset within the chunk (fully-masked prefix), w = width,
    scol = column offset of the block in the chunk's S tile."""
    blks = []
    scol = 0
    for kt in range(4 * qc + 4):
        off = max(0, kt * 128 - qc * 512)
        w = 512 - off
        blks.append((kt, off, w, scol))
        scol += w
    return blks


def split_multi_waits(nc):
    """This walrus build allows 1 sync wait per instruction (2 for
    EventSemaphore). Tile attaches more on some instructions (tail drain,
    collective-adjacent DMAs); hoist the extras onto same-engine NoOps."""
    for f in nc.m.functions:
        for bb in f.blocks:
            new_insts = []
            changed = False
            for ins in bb.instructions:
                si = ins.sync_info
                cap = 2 if type(ins).__name__ == "InstEventSemaphore" else 1
                if si is not None and len(si.on_wait) > cap:
                    waits = list(si.on_wait)
                    for k, w in enumerate(waits[cap:]):
                        new_insts.append(mybir.InstNoOp(
                            name=f"{ins.name}-wsplit{k}", ins=[], outs=[],
                            engine=ins.engine,
                            sync_info=bass_rust.SyncInfo(on_wait=[w], on_update=[]),
                        ))
                    ins.sync_info = bass_rust.SyncInfo(
                        on_wait=waits[:cap], on_update=list(si.on_update))
                    changed = True
                new_insts.append(ins)
            if changed:
                bb.instructions.clear()
                for i2 in new_insts:
                    bb.add_instruction(i2)


def build(fix_waits=True, dummy_cc=True):
    nc = bass.Bass()
    xT = nc.declare_dram_parameter("xT", [D, T], BF, isOutput=False)
    wqkvT = nc.declare_dram_parameter("wqkvT", [D, 3 * ES], BF, isOutput=False)
    # per-head stacked trig tables: rows 0:64 = even-col table, 64:128 = odd
    cc_p = [nc.declare_dram_parameter(f"cc{h}", [P, L], BF, isOutput=False)
            for h in range(HPC)]
    ss_p = [nc.declare_dram_parameter(f"ss{h}", [P, L], BF, isOutput=False)
            for h in range(HPC)]
    woT = nc.declare_dram_parameter("woT", [D, ES], BF, isOutput=False)
    out = nc.declare_dram_parameter("out", [ES, T], BF, isOutput=True)

    o_bounce, ag_o = {}, {}
    for b, pieces in AG_PIECES.items():
        for (c0, c1) in pieces:
            w = (c1 - c0) * 512
            o_bounce[(b, c0)] = nc.dram_tensor(f"o_bounce{b}_{c0}", [ES, w], BF)
            ag_o[(b, c0)] = nc.dram_tensor(f"ag_o{b}_{c0}", [N_CORES * ES, w], BF,
                                           addr_space="Shared")
    rg = [list(range(N_CORES))]
    if dummy_cc:
        dummy_in = nc.dram_tensor("dummy_in", [1, 256], BF)
        dummy_out = nc.dram_tensor("dummy_ag", [N_CORES, 256], BF,
                                   addr_space="Shared")

    with TileContext(nc, pool_alloc_mode="queue") as tc:
        with (
            tc.tile_pool(name="const", bufs=1) as const_pool,
            tc.tile_pool(name="resident", bufs=1) as res_pool,
            tc.tile_pool(name="wo", bufs=1) as wo_pool,
            tc.tile_pool(name="vt", bufs=1) as vt_pool,
        ):
            if dummy_cc:
                # skew-absorbing tiny collective: aligns the 8 cores while
                # phase 1 computes, so the real AllGathers don't eat the skew
                zt = const_pool.tile([1, 256], BF, name="zt")
                nc.gpsimd.memset(zt[:, :], 0.0)
                nc.gpsimd.dma_start(out=dummy_in[:, :], in_=zt[:, :])
                nc.gpsimd.collective_compute(
                    "AllGather", mybir.AluOpType.bypass,
                    ins=[dummy_in[:]], outs=[dummy_out[:]],
                    replica_groups=rg)

            ident = const_pool.tile([P, P], BF, name="ident")
            make_identity(nc, ident[:, :])
            ones = const_pool.tile([P, P], BF, name="ones")
            nc.gpsimd.memset(ones[:, :], 1.0)
            tri = const_pool.tile([P, P], BF, name="tri")
            nc.gpsimd.memset(tri[:, :], 1.0)
            nc.gpsimd.affine_select(
                out=tri[:, :], in_=tri[:, :],
                compare_op=mybir.AluOpType.is_ge, fill=0.0, base=0,
                pattern=[[1, P]], channel_multiplier=-1)

            # resident through phases 1-2
            qt_sb = res_pool.tile([P, HPC * T], BF, name="qt_sb")   # [hd', h*T+t]
            kt_sb = res_pool.tile([P, HPC * T], BF, name="kt_sb")
            v_sb = res_pool.tile([P, N_TT * ES], BF, name="v_sb")   # [t%128, tt*ES+e]
            woT_sb = wo_pool.tile([P, N_DT * ES], BF, name="woT_sb")

            # ---------------- phase 1: transposed QKV + RoPE ----------------
            # eb order: q-h0, q-h1, k-h0, k-h1, v-0, v-1
            with (
                tc.tile_pool(name="wq", bufs=1) as wq_pool,
                tc.tile_pool(name="xt", bufs=1) as xt_pool,
                tc.tile_pool(name="rsc", bufs=1) as rsc_pool,
                tc.tile_pool(name="psG", bufs=2, space="PSUM") as psG,
            ):
                wt_sb = wq_pool.tile([P, N_DT * 3 * ES], BF, name="wt_sb")
                cc_sb = [wq_pool.tile([P, L], BF, name=f"cc{h}_sb")
                         for h in range(HPC)]
                ss_sb = [wq_pool.tile([P, L], BF, name=f"ss{h}_sb")
                         for h in range(HPC)]
                xt_sb = xt_pool.tile([P, N_DT * 2048], BF, name="xt_sb")
                vt_sb = vt_pool.tile([P, HPC * T], BF,
                                     name="vt_sb")  # [e, eb*T + t]

                # DMA priority: x^T th0 tiles + weights interleaved, then trig
                for dt in range(N_DT):
                    nc.sync.dma_start(
                        out=xt_sb[:, dt * 2048:(dt + 1) * 2048],
                        in_=xT[dt * P:(dt + 1) * P, 0:2048])
                    nc.sync.dma_start(
                        out=wt_sb[:, dt * 3 * ES:(dt + 1) * 3 * ES],
                        in_=wqkvT[dt * P:(dt + 1) * P, :])
                for h in range(HPC):
                    nc.sync.dma_start(out=cc_sb[h][:, :], in_=cc_p[h][:, :])
                    nc.sync.dma_start(out=ss_sb[h][:, :], in_=ss_p[h][:, :])

                def rope_drain(gp, dst, h, th):
                    cc, ss = cc_sb[h], ss_sb[h]
                    dcol = slice(h * T + th * 2048, h * T + (th + 1) * 2048)
                    e_ps, o_ps = gp[0:64, :], gp[64:128, :]
                    t1 = rsc_pool.tile([64, 2048], FP, name="t1", tag="t1")
                    t2 = rsc_pool.tile([64, 2048], FP, name="t2", tag="t2")
                    nc.vector.tensor_tensor(t1[:, :], e_ps, cc[0:64, :],
                                            op=mybir.AluOpType.mult)
                    nc.vector.tensor_tensor(t2[:, :], o_ps, ss[0:64, :],
                                            op=mybir.AluOpType.mult)
                    nc.vector.tensor_tensor(dst[0:64, dcol], t1[:, :], t2[:, :],
                                            op=mybir.AluOpType.subtract)
                    t3 = rsc_pool.tile([64, 2048], FP, name="t3", tag="t1")
                    t4 = rsc_pool.tile([64, 2048], FP, name="t4", tag="t2")
                    nc.vector.tensor_tensor(t3[:, :], o_ps, cc[64:128, :],
                                            op=mybir.AluOpType.mult)
                    nc.vector.tensor_tensor(t4[:, :], e_ps, ss[64:128, :],
                                            op=mybir.AluOpType.mult)
                    nc.vector.tensor_tensor(dst[64:128, dcol], t3[:, :], t4[:, :],
                                            op=mybir.AluOpType.add)

                # th0: v first (tables can lag); th1: end with v so the
                # transpose stage isn't gated on a RoPE drain
                EB_ORDERS = [[4, 5, 0, 1, 2, 3], [4, 0, 1, 2, 3, 5]]
                for th in range(2):
                    for i, ebi in enumerate(EB_ORDERS[th]):
                        gp = psG.tile([P, 2048], FP, name="gp", tag="gp")
                        for dt in range(N_DT):
                            lhsT = wt_sb[:, dt * 3 * ES + ebi * P:
                                         dt * 3 * ES + (ebi + 1) * P]
                            for c in range(4):
                                nc.tensor.matmul(
                                    gp[:, c * 512:(c + 1) * 512], lhsT,
                                    xt_sb[:, dt * 2048 + c * 512:
                                          dt * 2048 + (c + 1) * 512],
                                    start=(dt == 0), stop=(dt == N_DT - 1))
                            if th == 0 and i == 5:
                                nc.sync.dma_start(
                                    out=xt_sb[:, dt * 2048:(dt + 1) * 2048],
                                    in_=xT[dt * P:(dt + 1) * P, 2048:4096])
                        if ebi < 2:
                            rope_drain(gp, qt_sb, ebi, th)
                        elif ebi < 4:
                            rope_drain(gp, kt_sb, ebi - 2, th)
                        else:
                            eb2 = ebi - 4
                            nc.scalar.copy(
                                vt_sb[:, eb2 * T + th * 2048:
                                      eb2 * T + (th + 1) * 2048],
                                gp[:, :])

            # ---------------- phases 2+3 (interleaved) ----------------
            for dt in range(N_DT):
                nc.sync.dma_start(out=woT_sb[:, dt * ES:(dt + 1) * ES],
                                  in_=woT[dt * P:(dt + 1) * P, :])

            with (
                tc.tile_pool(name="pS", bufs=2) as pS,
                tc.tile_pool(name="pAcc", bufs=2) as pAcc,
                tc.tile_pool(name="p2ob", bufs=2) as p2ob,
                tc.tile_pool(name="p3x", bufs=2) as p3x,
                tc.tile_pool(name="p3o", bufs=2) as p3o,
                tc.tile_pool(name="psW", bufs=2, space="PSUM") as psW,
                tc.tile_pool(name="psSm", bufs=1, space="PSUM") as psSm,
                tc.tile_pool(name="psO", bufs=1, space="PSUM") as psO,
            ):
                def phase2_chunk(b, qc, ob_sb):
                    blks = chunk_blocks(qc)
                    pairs = [tuple(blks[i:i + 2])
                             for i in range(0, len(blks), 2)]
                    CW = blks[-1][3] + blks[-1][2]
                    nkt = 4 * qc + 4
                    S = {h: pS.tile([P, CW], BF, name=f"S{h}", tag=f"S{h}")
                         for h in range(HPC)}
                    o_ps = {h: psO.tile([P, 512], FP, name=f"o{h}",
                                        tag=f"o{h}")
                            for h in range(HPC)}
                    sm = {h: psSm.tile([P, 512], FP, name=f"sm{h}",
                                       tag=f"sm{h}")
                          for h in range(HPC)}

                    def pv_pair(pr, h):
                        # PV accumulation + softmax-sum accumulation (both on
                        # PE; sums via all-ones matmuls into a PSUM bank)
                        for (kt, off, w, scol) in pr:
                            nc.tensor.matmul(
                                sm[h][:, off:],
                                ones[:, :],
                                S[h][:, scol:scol + w],
                                start=(kt == 0), stop=(kt == nkt - 1))
                            nc.tensor.matmul(
                                o_ps[h][:, off:],
                                v_sb[:, (b * N_LT + kt) * ES + h * HD:
                                     (b * N_LT + kt) * ES + (h + 1) * HD],
                                S[h][:, scol:scol + w],
                                start=(kt == 0), stop=(kt == nkt - 1))

                    for i, pr in enumerate(pairs):
                        for h in range(HPC):
                            qoff = h * T + b * L
                            sp = psW.tile([P, 1024], FP, name="sp", tag="w")
                            spo = 0
                            for (kt, off, w, scol) in pr:
                                nc.tensor.matmul(
                                    sp[:, spo:spo + w],
                                    kt_sb[:, qoff + kt * P:
                                          qoff + (kt + 1) * P],
                                    qt_sb[:, qoff + qc * 512 + off:
                                          qoff + (qc + 1) * 512],
                                    start=True, stop=True)
                                spo += w
                            wtot = sum(pr_b[2] for pr_b in pr)
                            nc.scalar.activation(
                                S[h][:, pr[0][3]:pr[0][3] + wtot],
                                sp[:, 0:wtot],
                                mybir.ActivationFunctionType.Exp, scale=SCALE)
                            # diagonal blocks: zero the masked (k>q) triangle
                            for (kt, off, w, scol) in pr:
                                if kt >= 4 * qc:
                                    nc.vector.tensor_tensor(
                                        S[h][:, scol:scol + P],
                                        S[h][:, scol:scol + P],
                                        tri[:, :], op=mybir.AluOpType.mult)
                        if i > 0:
                            for h in range(HPC):
                                pv_pair(pairs[i - 1], h)
                    for h in range(HPC):
                        pv_pair(pairs[-1], h)

                    # finalize: 1/sum = exp(-ln(sum)), rescale drain,
                    # per-chunk bounce DMA (parallel across queues)
                    lsm, rec = {}, {}
                    for h in range(HPC):
                        lsm[h] = pAcc.tile([P, 512], FP, name=f"lsm{h}",
                                           tag=f"lsm{h}")
                        nc.scalar.activation(lsm[h][:, :], sm[h][:, :],
                                             mybir.ActivationFunctionType.Ln)
                    for h in range(HPC):
                        rec[h] = pAcc.tile([P, 512], FP, name=f"rec{h}",
                                           tag=f"rec{h}")
                        nc.scalar.activation(rec[h][:, :], lsm[h][:, :],
                                             mybir.ActivationFunctionType.Exp,
                                             scale=-1.0)
                    (c0, c1) = piece_of(b, qc)
                    for h in range(HPC):
                        nc.vector.tensor_tensor(
                            ob_sb[:, h * L + qc * 512:h * L + (qc + 1) * 512],
                            o_ps[h][:, :], rec[h][:, :],
                            op=mybir.AluOpType.mult)
                        nc.sync.dma_start(
                            out=o_bounce[(b, c0)][h * HD:(h + 1) * HD,
                                                  (qc - c0) * 512:
                                                  (qc - c0 + 1) * 512],
                            in_=ob_sb[:, h * L + qc * 512:
                                      h * L + (qc + 1) * 512])

                def ag_fire(b, c0):
                    nc.gpsimd.collective_compute(
                        "AllGather", mybir.AluOpType.bypass,
                        ins=[o_bounce[(b, c0)][:]],
                        outs=[ag_o[(b, c0)][:]],
                        replica_groups=rg)

                def tr_group(th, eb2, tg):
                    tr = psW.tile([P, 512], BF, name="tr", tag="w")
                    for j in range(4):
                        tt_g = th * N_LT + tg * 4 + j
                        nc.tensor.transpose(
                            tr[:, j * P:(j + 1) * P],
                            vt_sb[:, eb2 * T + tt_g * P:
                                  eb2 * T + (tt_g + 1) * P],
                            ident[:, :])
                    for j in range(4):
                        tt_g = th * N_LT + tg * 4 + j
                        nc.vector.tensor_copy(
                            v_sb[:, tt_g * ES + eb2 * P:
                                 tt_g * ES + (eb2 + 1) * P],
                            tr[:, j * P:(j + 1) * P])

                def p3_load(b, c0, tch):
                    ot = p3x.tile([P, N_DT * 512], BF, name="ot", tag="ot")
                    for dt in range(N_DT):
                        nc.sync.dma_start(
                            out=ot[:, dt * 512:(dt + 1) * 512],
                            in_=ag_o[(b, c0)][dt * P:(dt + 1) * P,
                                              (tch - c0) * 512:
                                              (tch - c0 + 1) * 512])
                    return ot

                def p3_mm(b, tch, ot):
                    t0 = b * L + tch * 512
                    for et in range(HPC):
                        f_ps = psW.tile([P, 1024], FP, name="f_ps", tag="w")
                        for dt in range(N_DT):
                            nc.tensor.matmul(
                                f_ps[:, 0:512],
                                woT_sb[:, dt * ES + et * P:
                                       dt * ES + (et + 1) * P],
                                ot[:, dt * 512:(dt + 1) * 512],
                                start=(dt == 0), stop=(dt == N_DT - 1))
                        f_sb = p3o.tile([P, 512], BF, name="f_sb", tag="f")
                        nc.vector.tensor_copy(f_sb[:, :], f_ps[:, 0:512])
                        nc.sync.dma_start(
                            out=out[et * P:(et + 1) * P, t0:t0 + 512],
                            in_=f_sb[:, :])

                # ---- block 1: phase2(b=0), v transposes interleaved ----
                ob_tiles = {}
                for tg in range(4):           # batch-0 v tiles first
                    tr_group(0, 0, tg)
                    tr_group(0, 1, tg)
                ob_tiles[0] = p2ob.tile([P, HPC * L], BF,
                                        name="ob_sb", tag="ob")
                for qc in range(4):
                    phase2_chunk(0, qc, ob_tiles[0])
                    tr_group(1, 0, qc)        # batch-1 v tiles, spread out
                    tr_group(1, 1, qc)
                ag_fire(0, 0)

                # ---- block 2: phase2(b=1) with Wo pieces interleaved ----
                ob_tiles[1] = p2ob.tile([P, HPC * L], BF,
                                        name="ob_sb", tag="ob")
                phase2_chunk(1, 0, ob_tiles[1])
                phase2_chunk(1, 1, ob_tiles[1])
                ag_fire(1, 0)
                ot00 = p3_load(0, 0, 0)
                ot01 = p3_load(0, 0, 1)
                phase2_chunk(1, 2, ob_tiles[1])
                p3_mm(0, 0, ot00)
                p3_mm(0, 1, ot01)
                ot02 = p3_load(0, 0, 2)
                ot03 = p3_load(0, 0, 3)
                phase2_chunk(1, 3, ob_tiles[1])
                ag_fire(1, 2)
                p3_mm(0, 2, ot02)
                p3_mm(0, 3, ot03)
                for tch in (0, 1):
                    ot = p3_load(1, 0, tch)
                    p3_mm(1, tch, ot)
                for tch in (2, 3):
                    ot = p3_load(1, 2, tch)
                    p3_mm(1, tch, ot)

    if fix_waits:
        split_multi_waits(nc)
    return nc


def make_in_maps(x, cos, sin, Wqkv, Wo):
    bf = ml_dtypes.bfloat16
    xT_full = np.ascontiguousarray(
        np.asarray(x).reshape(T, D).T).astype(bf)
    # q/k row permutation: head-major, evens then odds
    perm = []
    for h in range(HPC):
        perm.extend(h * HD + 2 * np.arange(64))
        perm.extend(h * HD + 2 * np.arange(64) + 1)
    perm = np.asarray(perm)
    in_maps = []
    cosA, sinA = np.asarray(cos), np.asarray(sin)
    for c in range(N_CORES):
        cols = slice(c * ES, (c + 1) * ES)
        wq = Wqkv[c * ES:(c + 1) * ES, :][perm]
        wk = Wqkv[D + c * ES: D + (c + 1) * ES, :][perm]
        wv = Wqkv[2 * D + c * ES: 2 * D + (c + 1) * ES, :]
        w_c = np.concatenate([wq, wk, wv], axis=0)
        m = {
            "xT": xT_full,
            "wqkvT": np.ascontiguousarray(w_c.T.astype(bf)),
            "woT": np.ascontiguousarray(Wo[cols, :].T.astype(bf)),
        }
        for h in range(HPC):
            base = c * ES + h * HD
            ce = cosA[:, base + 2 * np.arange(64)].T      # [64, L]
            co = cosA[:, base + 2 * np.arange(64) + 1].T
            se = sinA[:, base + 2 * np.arange(64)].T
            so = sinA[:, base + 2 * np.arange(64) + 1].T
            m[f"cc{h}"] = np.ascontiguousarray(
                np.concatenate([ce, co], axis=0)).astype(bf)
            m[f"ss{h}"] = np.ascontiguousarray(
                np.concatenate([se, so], axis=0)).astype(bf)
        in_maps.append(m)
    return in_maps


def gather_out(res):
    pieces = [np.asarray(res.results[c]["out"]).astype(np.float32).T
              for c in range(N_CORES)]
    return np.concatenate(pieces, axis=1).reshape(B, L, D)


_cache = {}


def kernel(x, cos, sin, Wqkv, Wo):
    from concourse.bass_utils import run_bass_kernel_spmd
    x = np.asarray(x, dtype=np.float32)
    cos = np.asarray(cos, dtype=np.float32)
    sin = np.asarray(sin, dtype=np.float32)
    Wqkv = np.asarray(Wqkv, dtype=np.float32)
    Wo = np.asarray(Wo, dtype=np.float32)
    if "nc" not in _cache:
        _cache["nc"] = build()
    nc = _cache["nc"]
    in_maps = make_in_maps(x, cos, sin, Wqkv, Wo)
    res = run_bass_kernel_spmd(nc, in_maps, core_ids=list(range(N_CORES)))
    return gather_out(res)
